# revision 1
# baseline (speedup 1.0000x reference)
"""BuzzLoss Trainium2 kernel.

Math (telescoped form of the reference):
    excl[t] = prod_{s<t} (1 - conf[s])          (exclusive cumprod)
    score_b = sum_t excl[b,t] * da[b,t]
    da[b,0] = acc[b,0];  da[b,t] = acc[b,t] - acc[b,t-1]
    out = -mean_b score_b

Derivation: buzz[t] = conf[t]*excl[t] = excl[t] - excl[t+1] telescopes, and
the correction term (1 - sum buzz) * acc[T-1] = cum[T-1]*acc[T-1] cancels
against the boundary of the summation-by-parts.  Equivalently
score_b = sum_t excl[t]*acc[t] - sum_{t>=1} excl[t]*acc[t-1] ("pos/neg"
form) — used for the last tiles so both fused passes run on DVE with no
GPSIMD dependency in the kernel tail.

Sharding: pure data parallel — batch 8192 split across 8 NeuronCores (1024
rows each).  Each core emits per-row partial sums with per-column signs;
the host combines, takes the mean, and negates.  No collectives.

Per 128-row tile on-chip:
    ACT   : nb = 1 - conf                    (activation Copy, scale=-1, bias=1)
    DVE   : excl = hardware prefix scan      (tensor_tensor_scan, mult — fp32
            recurrence state, bf16 output, whole cumprod in one instruction)
    GPSIMD: da = shifted subtract of acc     (bf16 out; da in {-1,0,1} exact)
    DVE   : res column = fused mul+row-sum   (scalar_tensor_tensor + accum_out;
            bf16 operands enable the DVE 2x packed mode, fp32 accumulator)
The t=0 boundary term (= acc[b,0]) is added by the host from the raw input.

DMA: all loads on the SP HWDGE ring; early tiles conf-ahead interleaved; the
LAST TWO tiles arrive with conf and acc interleaved in halves (chained scans,
half-width da/stt) so each final arrival's follow-up work is short and lands
on a different engine.
"""

import numpy as np

import concourse.bacc as bacc
import concourse.mybir as mybir
import concourse.tile as tile
from concourse.bass_utils import run_bass_kernel_spmd

B, T = 8192, 1024
N_CORES = 8
ROWS = B // N_CORES  # rows per core
P = 128  # SBUF partitions
NTILES = ROWS // P  # row-tiles per core

H = T // 2
Q = T // 4

# (kind, tile, seg) load order: conf-ahead interleave for the early tiles;
# the LAST TWO tiles arrive with conf and acc interleaved in halves/quarters
# so the tail work after each arrival splits across ACT (nb), DVE
# (scan+stt), and GPSIMD (da) instead of piling onto one engine.
LOAD_ORDER = [
    ("c", 0, (0, T)), ("a", 0, (0, T)),
    ("c", 1, (0, T)), ("c", 2, (0, T)), ("a", 1, (0, T)),
    ("c", 3, (0, T)), ("a", 2, (0, T)),
    ("c", 4, (0, T)), ("a", 3, (0, H + 1)), ("a", 3, (H + 1, T)),
    ("c", 5, (0, T)), ("a", 4, (0, H + 1)), ("a", 4, (H + 1, T)),
    ("a", 5, (0, H + 1)), ("a", 5, (H + 1, T)),
    ("c", 6, (0, H)), ("c", 6, (H, T)), ("a", 6, (0, H + 1)), ("a", 6, (H + 1, T)),
    ("c", 7, (0, H)), ("c", 7, (H, T)), ("a", 7, (0, H + 1)), ("a", 7, (H + 1, T)),
]

# per-tile compute plan: ("da", segs[, scan_segs]) or ("pn", segs[, scan_segs])
#  "da": GPSIMD shifted-subtract + one DVE stt per seg (one +1 column each)
#  "pn": DVE stt pos and neg per seg (one +1 and one -1 column each)
# scan_segs (over nb indices 0..T-2) chain the hardware scan so excl is
# produced incrementally as conf segments land.
# stt segs use boundary H+1 so each bf16-shifted slice starts 4B-aligned
# AND each seg's acc reads stay within one acc DMA segment.
PLAN = {
    0: ("da", [(0, T)]),
    1: ("da", [(0, T)]),
    2: ("da", [(0, T)]),
    3: ("da", [(0, H + 1), (H + 1, T)], [(0, H), (H, T - 1)]),
    4: ("da", [(0, H + 1), (H + 1, T)], [(0, H), (H, T - 1)]),
    5: ("da", [(0, H + 1), (H + 1, T)], [(0, H), (H, T - 1)]),
    6: ("da", [(0, H + 1), (H + 1, T)], [(0, H), (H, T - 1)]),
    7: ("da", [(0, H + 1), (H + 1, T)], [(0, H), (H, T - 1)]),
}

f32 = mybir.dt.float32
bf16 = mybir.dt.bfloat16


def _n_cols(plan):
    n = 0
    for entry in plan.values():
        mode, segs = entry[0], entry[1]
        n += len(segs) * (2 if mode == "pn" else 1)
    return n


def _col_signs(plan):
    signs = []
    for j in sorted(plan):
        entry = plan[j]
        mode, segs = entry[0], entry[1]
        for _ in segs:
            signs.append(1.0)
            if mode == "pn":
                signs.append(-1.0)
    return np.array(signs, dtype=np.float64)


NCOLS = _n_cols(PLAN)
COL_SIGNS = _col_signs(PLAN)

_CACHE = {}


def _emit_pipeline(nc, io_pool, work_pool, res, conf_r, acc_r, rep, plan, load_order):
    Alu = mybir.AluOpType
    ct, at = {}, {}
    for kind, j, (a, b) in load_order:
        if kind == "c":
            if j not in ct:
                ct[j] = io_pool.tile(
                    [P, T], f32, tag="conf", name=f"conf_t{rep}_{j}"
                )
            nc.sync.dma_start(ct[j][:, a:b], conf_r[j][:, a:b])
        else:
            if j not in at:
                at[j] = io_pool.tile([P, T], f32, tag="acc", name=f"acc_t{rep}_{j}")
            nc.sync.dma_start(at[j][:, a:b], acc_r[j][:, a:b])

    col = 0
    for j in sorted(plan):
        conf_t = ct[j]
        acc_t = at[j]
        entry = plan[j]
        mode, segs = entry[0], entry[1]
        scan_segs = entry[2] if len(entry) > 2 else [(0, T - 1)]

        # nb = 1 - conf (ScalarE); excl = chained prefix scan (DVE).
        # excl/da/scr are bf16 in SHIFTED layout (buf[i] = value at t=i+1)
        # so the stt runs in the DVE 2x_1P mode (2 elem/cycle) with slices
        # starting 4B-aligned.  The scan's recurrence state stays fp32 in
        # hardware; only the stored excl is bf16 (rounding ~0.4% on values
        # that decay geometrically — immaterial vs the 2e-2 budget).
        # excl[0] (== 1.0) is never materialized: the t=0 score term equals
        # acc[b,0], which the host adds from the raw input (see kernel()).
        # scan seg [a,b) over nb indices writes shifted excl[a:b] with
        # initial = excl[a-1] (the t=a cumprod).
        nb = work_pool.tile([P, T], f32, tag="nb")
        excl = work_pool.tile([P, T], bf16, tag="excl")
        for a, b in scan_segs:
            nc.scalar.activation(
                nb[:, a:b],
                conf_t[:, a:b],
                mybir.ActivationFunctionType.Copy,
                bias=1.0,
                scale=-1.0,
            )
            nc.vector.tensor_tensor_scan(
                excl[:, a:b],
                nb[:, a:b],
                nb[:, a:b],
                1.0 if a == 0 else excl[:, a - 1 : a],
                Alu.mult,
                Alu.bypass,
            )

        if mode == "da":
            da = work_pool.tile([P, T], bf16, tag="da")
            scr = work_pool.tile([P, T], bf16, tag="scr")
            for a, b in segs:
                a1 = max(a, 1)
                # shifted: da[i] = acc[i+1] - acc[i]; slice [a1-1 : b-1]
                nc.gpsimd.tensor_sub(
                    da[:, a1 - 1 : b - 1],
                    acc_t[:, a1:b],
                    acc_t[:, a1 - 1 : b - 1],
                )
                nc.vector.scalar_tensor_tensor(
                    scr[:, a1 - 1 : b - 1],
                    excl[:, a1 - 1 : b - 1],
                    1.0,
                    da[:, a1 - 1 : b - 1],
                    Alu.bypass,
                    Alu.mult,
                    accum_out=res[:, col : col + 1],
                )
                col += 1
        else:  # pos/neg, all DVE
            scr = work_pool.tile([P, T], f32, tag="scr")
            for a, b in segs:
                a1 = max(a, 1)
                nc.vector.scalar_tensor_tensor(
                    scr[:, a1:b],
                    excl[:, a1:b],
                    1.0,
                    acc_t[:, a1:b],
                    Alu.bypass,
                    Alu.mult,
                    accum_out=res[:, col : col + 1],
                )
                col += 1
                nc.vector.scalar_tensor_tensor(
                    scr[:, a1:b],
                    excl[:, a1:b],
                    1.0,
                    acc_t[:, a1 - 1 : b - 1],
                    Alu.bypass,
                    Alu.mult,
                    accum_out=res[:, col : col + 1],
                )
                col += 1


def build_bass(reps: int = 1, plan=None, load_order=None):
    plan = plan or PLAN
    load_order = load_order or LOAD_ORDER
    ncols = _n_cols(plan)
    nc = bacc.Bacc("TRN2", target_bir_lowering=False, debug=False)
    conf = nc.declare_dram_parameter("confidences", [ROWS, T], f32, isOutput=False)
    acc = nc.declare_dram_parameter("accuracies", [ROWS, T], f32, isOutput=False)
    out = nc.declare_dram_parameter("partials", [P, ncols], f32, isOutput=True)

    conf_r = conf.rearrange("(n p) t -> n p t", p=P)
    acc_r = acc.rearrange("(n p) t -> n p t", p=P)

    with tile.TileContext(nc) as tc:
        with (
            tc.tile_pool(name="io", bufs=NTILES) as io_pool,
            tc.tile_pool(name="work", bufs=8) as work_pool,
            tc.tile_pool(name="res", bufs=1) as res_pool,
        ):
            res = res_pool.tile([P, ncols], f32)
            for rep in range(reps):
                _emit_pipeline(
                    nc, io_pool, work_pool, res, conf_r, acc_r, rep, plan, load_order
                )
            nc.sync.dma_start(out[:], res[:])
    nc.compile()
    return nc


def make_in_maps(confidences: np.ndarray, accuracies: np.ndarray):
    conf = np.ascontiguousarray(np.asarray(confidences, dtype=np.float32))
    acc = np.ascontiguousarray(np.asarray(accuracies, dtype=np.float32))
    return [
        {
            "confidences": conf[i * ROWS : (i + 1) * ROWS],
            "accuracies": acc[i * ROWS : (i + 1) * ROWS],
        }
        for i in range(N_CORES)
    ]


def reduce_partials(results, accuracies) -> np.ndarray:
    # device partials + the t=0 boundary term sum_b acc[b, 0]
    total = float(np.sum(np.asarray(accuracies)[:, 0], dtype=np.float64))
    for r in results:
        p = r["partials"].astype(np.float64)
        total += float(np.dot(p.sum(axis=0), COL_SIGNS))
    return np.asarray(-(total / B), dtype=np.float32)


def kernel(confidences: np.ndarray, accuracies: np.ndarray) -> np.ndarray:
    if "nc" not in _CACHE:
        _CACHE["nc"] = build_bass()
    nc = _CACHE["nc"]
    results = run_bass_kernel_spmd(
        nc, make_in_maps(confidences, accuracies), list(range(N_CORES))
    ).results
    return reduce_partials(results, accuracies)

